# revision 1
# baseline (speedup 1.0000x reference)
"""JumpGCN-v2 (GCNII + JK-max + MLP branch) on 8 Trainium2 NeuronCores.

Sharding: nodes row-sharded across 8 cores; edges partitioned by destination
node (segment-sum stays local); per-layer halo exchange = AllGather of the
updated h shards into a full gather table in each core's HBM; weights
replicated.

spmm per core/layer: dma_gather of h[src] rows (the src index space is split
into 4 buckets of 25000 rows so indices fit int16), per-edge weighting on the
scalar engine, and segment-sum via a one-hot matmul on the tensor engine
(one-hot built with a single is_equal DVE op per 4 chunks of 128 edges).
"""
import math
import os

import numpy as np

import concourse.bass as bass
import concourse.bacc as bacc
import concourse.mybir as mybir
import concourse.tile as tile
from concourse import bass_utils
from concourse.masks import make_identity

F32 = mybir.dt.float32
I16 = mybir.dt.int16
AF = mybir.ActivationFunctionType
ALU = mybir.AluOpType

NCORES = 8
N = 100000
D_IN = 128
H = 64
L = 4
ALPHA = 0.1
LAMDA = 1.0
NSH = N // NCORES            # 12500 nodes per core
NT = math.ceil(NSH / 128)    # 98 dst tiles
NSHP = NT * 128              # 12544 padded shard rows
NBUCK = 4
BUCK = N // NBUCK            # 25000 table rows per src bucket
PIECE = 2048                 # edges per dma_gather
LN_EPS = 1e-5
THETA = [float(np.log(LAMDA / (l + 1) + 1.0)) for l in range(L)]
LAST_EXEC_NS = 0


# ---------------------------------------------------------------- host prep
def _prep_edges(edge_index, edge_weight):
    """Partition/pad the edge list. Returns per-core streams plus the shared
    schedule (chunk counts per (bucket, dst-tile), identical for all cores)."""
    src = np.asarray(edge_index[0], np.int64)
    dst = np.asarray(edge_index[1], np.int64)
    w = np.asarray(edge_weight, np.float32)

    core = dst // NSH
    dstl = dst - core * NSH
    t = dstl // 128
    dloc = (dstl - t * 128).astype(np.float32)
    b = src // BUCK
    sidx = (src - b * BUCK).astype(np.int16)

    key = (core * NBUCK + b) * NT + t
    order = np.argsort(key, kind="stable")
    counts = np.bincount(key, minlength=NCORES * NBUCK * NT).reshape(
        NCORES, NBUCK, NT
    )
    # shared schedule: chunks per (b, t) = max over cores
    g_bt = -(-counts.max(axis=0) // 128)          # [NBUCK, NT] chunks
    g_bt = np.maximum(g_bt, 1)
    pad_bt = g_bt * 128                            # padded group length
    blen = pad_bt.sum(axis=1)                      # per-bucket stream length
    base_bt = np.zeros((NBUCK, NT), np.int64)
    for bb in range(NBUCK):
        base_bt[bb, 1:] = np.cumsum(pad_bt[bb])[:-1]

    # per-edge position within its (core, b, t) group
    group_start = np.zeros(NCORES * NBUCK * NT, np.int64)
    cflat = counts.reshape(-1)
    group_start[1:] = np.cumsum(cflat)[:-1]
    skey = key[order]
    pos_in_group = np.arange(len(order)) - group_start[skey]
    bt = b[order] * NT + t[order]
    stream_pos = base_bt.reshape(-1)[bt] + pos_in_group

    cores_data = []
    for c in range(NCORES):
        sel = order[core[order] == c]
        posc = stream_pos[core[order] == c]
        bc = b[sel]
        idx_s, w_s, d_s = [], [], []
        for bb in range(NBUCK):
            n_ = int(blen[bb])
            ia = np.zeros(n_, np.int16)
            wa = np.zeros(n_, np.float32)
            da = np.zeros(n_, np.float32)
            m = bc == bb
            p = posc[m]
            ia[p] = sidx[sel[m]]
            wa[p] = w[sel[m]]
            da[p] = dloc[sel[m]]
            # wrapped layouts
            iw = np.tile(ia.reshape(-1, 16).T, (8, 1))          # [128, n/16]
            ww = wa.reshape(-1, 128).T.copy()                    # [128, n/128]
            dw = da.reshape(-1, 128).T.copy()
            idx_s.append(iw)
            w_s.append(ww)
            d_s.append(dw)
        cores_data.append((idx_s, w_s, d_s))
    return cores_data, g_bt, blen


# ---------------------------------------------------------------- bass build
def _build(g_bt, blen):
    nc = bacc.Bacc("TRN2", target_bir_lowering=False, debug=False,
                   enable_asserts=True, num_devices=NCORES)

    din = {}
    def inp(name, shape, dt=F32):
        din[name] = nc.dram_tensor(name, list(shape), dt, kind="ExternalInput")
        return din[name]

    xsh = inp("xsh", [NSHP, D_IN])
    for bb in range(NBUCK):
        inp(f"idx{bb}", [128, int(blen[bb]) // 16], I16)
        inp(f"wst{bb}", [128, int(blen[bb]) // 128])
        inp(f"dst{bb}", [128, int(blen[bb]) // 128])
    proj_w = inp("proj_w", [D_IN, H])
    projb = inp("projb", [H, 1])
    mlp_w1 = inp("mlp_w1", [D_IN, H])
    b1 = inp("b1", [H, 1])
    mlp_w2 = inp("mlp_w2", [H, H])
    b2 = inp("b2", [H, 1])
    mlp_w3 = inp("mlp_w3", [H, 1])
    b3 = inp("b3", [1, 1])
    head_w = inp("head_w", [H, 1])
    headb = inp("headb", [1, 1])
    gcn_w = inp("gcn_w", [L, H, H])
    ln1g = inp("ln1g", [128, H])
    ln1b = inp("ln1b", [128, H])
    ln2g = inp("ln2g", [128, H])
    ln2b = inp("ln2b", [128, H])
    diota = inp("diota", [128, 128])

    out = nc.dram_tensor("out", [1, NSH], F32, kind="ExternalOutput")
    mrow_d = nc.dram_tensor("mrow_d", [1, NSHP], F32, kind="Internal")
    bounce = nc.dram_tensor("bounce", [NSH, H], F32, kind="Internal")
    table = nc.dram_tensor("table", [N, H], F32, kind="Internal",
                           addr_space="Shared")

    dbg_layers = int(os.environ.get("DBG_LAYERS", str(L)))
    dbg_gather = int(os.environ.get("DBG_GATHER", "1"))
    dbg_path = os.environ.get("DBG_PATH", "full")
    with tile.TileContext(nc) as tc:
        cst = tc.alloc_tile_pool(name="cst", bufs=1)
        big = tc.alloc_tile_pool(name="big", bufs=1)
        sb = tc.alloc_tile_pool(name="sb", bufs=3)
        msgp = tc.alloc_tile_pool(name="msgp", bufs=2)
        ohp = tc.alloc_tile_pool(name="ohp", bufs=6)
        mwp = tc.alloc_tile_pool(name="mwp", bufs=8)
        psA = tc.alloc_tile_pool(name="psA", bufs=2, space="PSUM")
        psB = tc.alloc_tile_pool(name="psB", bufs=2, space="PSUM")
        psC = tc.alloc_tile_pool(name="psC", bufs=1, space="PSUM")

        i128 = cst.tile([128, 128], F32)
        make_identity(nc, i128[:])
        i64 = cst.tile([64, 64], F32)
        make_identity(nc, i64[:])
        dio = cst.tile([128, 128], F32)
        nc.sync.dma_start(dio[:], diota[:, :])
        epst = cst.tile([128, 1], F32)
        nc.vector.memset(epst[:], LN_EPS)

        def load_const(t_, shape=None):
            tl = cst.tile(shape or list(t_.shape), t_.dtype, tag=t_.name)
            nc.sync.dma_start(tl[:], t_[:, :])
            return tl

        pw = load_const(proj_w)
        pb = load_const(projb)
        w1 = load_const(mlp_w1)
        b1t = load_const(b1)
        w2 = load_const(mlp_w2)
        b2t = load_const(b2)
        w3 = load_const(mlp_w3)
        b3t = load_const(b3)
        hw = load_const(head_w)
        hbt = load_const(headb)
        g1 = load_const(ln1g)
        be1 = load_const(ln1b)
        g2 = load_const(ln2g)
        be2 = load_const(ln2b)

        gw = [cst.tile([H, H], F32, tag=f"gw{l}", name=f"gw{l}")
              for l in range(L)]
        for l in range(L):
            nc.sync.dma_start(gw[l][:], gcn_w[l, :, :])

        h_all = big.tile([128, NT * H], F32)     # current h, node-major tiles
        h0s_all = big.tile([128, NT * H], F32)   # 0.1 * h0
        hi_all = big.tile([128, NT * H], F32)    # spmm accumulator
        xmax = big.tile([128, NT * H], F32)      # JK running max

        def ln_relu(m_sb, gt, bt_):
            """node-major layernorm + affine + relu on a [128, H] tile"""
            red = sb.tile([128, 1], F32, tag="red")
            nc.vector.reduce_sum(out=red[:], in_=m_sb[:],
                                 axis=mybir.AxisListType.X)
            nm = sb.tile([128, 1], F32, tag="nm")
            nc.vector.tensor_scalar_mul(nm[:], red[:], -1.0 / H)
            xc = sb.tile([128, H], F32, tag="xc")
            nc.vector.tensor_scalar_add(xc[:], m_sb[:], nm[:])
            sq = sb.tile([128, H], F32, tag="sq")
            nc.vector.tensor_tensor(out=sq[:], in0=xc[:], in1=xc[:],
                                    op=ALU.mult)
            var = sb.tile([128, 1], F32, tag="var")
            nc.vector.reduce_sum(out=var[:], in_=sq[:],
                                 axis=mybir.AxisListType.X)
            std = sb.tile([128, 1], F32, tag="std")
            nc.scalar.activation(std[:], var[:], AF.Sqrt, bias=epst[:],
                                 scale=1.0 / H)
            rs = sb.tile([128, 1], F32, tag="rs")
            nc.vector.reciprocal(rs[:], std[:])
            xn = sb.tile([128, H], F32, tag="xn")
            nc.vector.tensor_scalar_mul(xn[:], xc[:], rs[:])
            yg = sb.tile([128, H], F32, tag="yg")
            nc.vector.tensor_tensor(out=yg[:], in0=xn[:], in1=gt[:],
                                    op=ALU.mult)
            yb = sb.tile([128, H], F32, tag="yb")
            nc.vector.tensor_tensor(out=yb[:], in0=yg[:], in1=bt_[:],
                                    op=ALU.add)
            yr = sb.tile([128, H], F32, tag="yr")
            nc.scalar.activation(yr[:], yb[:], AF.Relu)
            return yr

        def transpose_128x64(src_ap):
            """[128, 64] -> [64, 128] sbuf"""
            ps = psB.tile([64, 128], F32, tag="tpB")
            nc.tensor.transpose(out=ps[:], in_=src_ap, identity=i128[:])
            st = sb.tile([64, 128], F32, tag="supT")
            nc.vector.tensor_copy(out=st[:], in_=ps[:])
            return st

        # ---------------- phase P: proj + MLP branch ----------------
        for t in range(NT):
            xt = sb.tile([128, D_IN], F32, tag="xt")
            nc.sync.dma_start(xt[:], xsh[t * 128:(t + 1) * 128, :])
            xps = psB.tile([128, 128], F32, tag="tpB")
            nc.tensor.transpose(out=xps[:], in_=xt[:], identity=i128[:])
            xT = sb.tile([128, 128], F32, tag="xT")
            nc.vector.tensor_copy(out=xT[:], in_=xps[:])

            # proj: h_T = proj_w.T @ x_T + b
            hps = psA.tile([64, 128], F32, tag="mmA")
            nc.tensor.matmul(out=hps[:], lhsT=pw[:], rhs=xT[:],
                             start=True, stop=True)
            # add proj bias (per-partition) while evacuating PSUM
            hTb = sb.tile([64, 128], F32, tag="hTb")
            nc.vector.tensor_scalar_add(hTb[:], hps[:], pb[:])
            hps2 = psB.tile([128, 64], F32, tag="tpB2")
            nc.tensor.matmul(out=hps2[:], lhsT=hTb[:], rhs=i64[:],
                             is_transpose=True)
            nc.vector.tensor_copy(out=h_all[:, t * H:(t + 1) * H], in_=hps2[:])
            nc.vector.tensor_scalar_mul(h0s_all[:, t * H:(t + 1) * H],
                                        hps2[:], ALPHA)

            # mlp layer 1
            mps = psA.tile([64, 128], F32, tag="mmA")
            nc.tensor.matmul(out=mps[:], lhsT=w1[:], rhs=xT[:],
                             start=True, stop=True)
            mT = sb.tile([64, 128], F32, tag="mT")
            nc.vector.tensor_scalar_add(mT[:], mps[:], b1t[:])
            mps2 = psB.tile([128, 64], F32, tag="tpB2")
            nc.tensor.matmul(out=mps2[:], lhsT=mT[:], rhs=i64[:],
                             is_transpose=True)
            m1 = sb.tile([128, H], F32, tag="m1")
            nc.vector.tensor_copy(out=m1[:], in_=mps2[:])
            y1 = ln_relu(m1, g1, be1)

            # mlp layer 2
            y1T = transpose_128x64(y1[:])
            m2ps = psA.tile([64, 128], F32, tag="mmA")
            nc.tensor.matmul(out=m2ps[:], lhsT=w2[:], rhs=y1T[:],
                             start=True, stop=True)
            m2T = sb.tile([64, 128], F32, tag="m2T")
            nc.vector.tensor_scalar_add(m2T[:], m2ps[:], b2t[:])
            m2ps2 = psB.tile([128, 64], F32, tag="tpB2")
            nc.tensor.matmul(out=m2ps2[:], lhsT=m2T[:], rhs=i64[:],
                             is_transpose=True)
            m2 = sb.tile([128, H], F32, tag="m2")
            nc.vector.tensor_copy(out=m2[:], in_=m2ps2[:])
            y2 = ln_relu(m2, g2, be2)

            # mlp layer 3 -> [1, 128] row
            y2T = transpose_128x64(y2[:])
            m3ps = psC.tile([1, 128], F32, tag="mmC")
            nc.tensor.matmul(out=m3ps[:], lhsT=w3[:], rhs=y2T[:],
                             start=True, stop=True)
            m3r = sb.tile([1, 128], F32, tag="m3r")
            nc.vector.tensor_scalar_add(m3r[:], m3ps[:], b3t[:])
            nc.sync.dma_start(mrow_d[:, t * 128:(t + 1) * 128], m3r[:])

        if dbg_layers < L or not dbg_gather or dbg_path != "full":
            nc.vector.memset(hi_all[:], 0.0)
            nc.vector.memset(xmax[:], 0.0)

        # ---------------- GCN layers ----------------
        for l in range(dbg_layers):
            # halo exchange: shard -> bounce -> AllGather -> table
            nc.sync.dma_start(
                bounce[0:97 * 128, :].rearrange("(t p) h -> p t h", p=128),
                h_all[:, 0:97 * H].rearrange("p (t h) -> p t h", h=H))
            nc.sync.dma_start(bounce[97 * 128:NSH, :],
                              h_all[0:NSH - 97 * 128, 97 * H:98 * H])
            nc.gpsimd.collective_compute(
                "AllGather", ALU.bypass,
                replica_groups=[list(range(NCORES))],
                ins=[bounce[:, :]], outs=[table[:, :]],
            )

            theta = THETA[l]
            for bb in range(NBUCK if dbg_gather else 0):
                n_b = int(blen[bb])
                chunks = [(t, j, jj) for t in range(NT)
                          for jj in [g_bt[bb][t]] for j in range(jj)]
                # stream pieces
                off = 0
                piece_tiles = []
                while off < n_b:
                    sz = min(PIECE, n_b - off)
                    it = msgp.tile([128, PIECE // 16], I16, tag="idx")
                    nc.sync.dma_start(it[:, :sz // 16],
                                      din[f"idx{bb}"][:, off // 16:(off + sz) // 16])
                    wt = msgp.tile([128, PIECE // 128], F32, tag="wt")
                    nc.sync.dma_start(wt[:, :sz // 128],
                                      din[f"wst{bb}"][:, off // 128:(off + sz) // 128])
                    dt_ = msgp.tile([128, PIECE // 128], F32, tag="dt")
                    nc.sync.dma_start(dt_[:, :sz // 128],
                                      din[f"dst{bb}"][:, off // 128:(off + sz) // 128])
                    mg = msgp.tile([128, PIECE // 128, H], F32, tag="mg")
                    nc.gpsimd.dma_gather(
                        mg[:, :sz // 128, :],
                        table[bb * BUCK:(bb + 1) * BUCK, :],
                        it[:, :sz // 16], sz, sz, H, elem_step=H,
                        single_packet=False)
                    piece_tiles.append((off, sz, it, wt, dt_, mg))
                    off += sz

                def piece_of(chunk_pos):
                    i = chunk_pos * 128 // PIECE
                    return piece_tiles[min(i, len(piece_tiles) - 1)]

                if dbg_path == "gather":
                    continue
                # one-hot groups of up to 4 chunks + matmuls
                ci = 0
                hi_ps = None
                pend = []  # (chunk_pos, oh_tile, oh_col, t, j, jj)
                total_chunks = n_b // 128
                # build in groups of 4 within a piece
                pos = 0
                oh_cache = {}
                for (t, j, jj) in chunks:
                    gidx = pos // 4
                    if gidx not in oh_cache:
                        g0 = gidx * 4
                        gsz = min(4, total_chunks - g0)
                        # all chunks of this group must be in same piece
                        poff, psz, _, _, pdt, _ = piece_of(g0)
                        loc0 = g0 - poff // 128
                        oh = ohp.tile([128, 4, 128], F32, tag="oh")
                        nc.vector.tensor_tensor(
                            out=oh[:, :gsz, :],
                            in0=pdt[:, loc0:loc0 + gsz].unsqueeze(2)
                                .to_broadcast([128, gsz, 128]),
                            in1=dio[:].unsqueeze(1).to_broadcast(
                                [128, gsz, 128]),
                            op=ALU.is_equal)
                        oh_cache = {gidx: oh}
                    oh = oh_cache[gidx]
                    poff, psz, _, pwt, _, pmg = piece_of(pos)
                    loc = pos - poff // 128
                    mw = mwp.tile([128, H], F32, tag="mw")
                    nc.scalar.activation(mw[:], pmg[:, loc, :], AF.Copy,
                                         scale=pwt[:, loc:loc + 1])
                    if dbg_path == "oh":
                        pos += 1
                        continue
                    if j == 0:
                        hi_ps = psA.tile([128, H], F32, tag="mmA")
                    nc.tensor.matmul(out=hi_ps[:],
                                     lhsT=oh[:, pos % 4, :], rhs=mw[:],
                                     start=(j == 0), stop=(j == jj - 1))
                    if j == jj - 1:
                        dstsl = hi_all[:, t * H:(t + 1) * H]
                        if bb == 0:
                            nc.vector.tensor_copy(out=dstsl, in_=hi_ps[:])
                        else:
                            nc.vector.tensor_tensor(out=dstsl, in0=dstsl,
                                                    in1=hi_ps[:], op=ALU.add)
                    pos += 1

            # ---------------- layer update ----------------
            for t in range(NT):
                sup = sb.tile([128, H], F32, tag="sup")
                nc.vector.tensor_scalar_mul(sup[:],
                                            hi_all[:, t * H:(t + 1) * H],
                                            1.0 - ALPHA)
                nc.vector.tensor_tensor(out=sup[:], in0=sup[:],
                                        in1=h0s_all[:, t * H:(t + 1) * H],
                                        op=ALU.add)
                supT = transpose_128x64(sup[:])
                gps = psA.tile([64, 128], F32, tag="mmA")
                nc.tensor.matmul(out=gps[:], lhsT=gw[l][:], rhs=supT[:],
                                 start=True, stop=True)
                t1 = sb.tile([64, 128], F32, tag="t1")
                nc.scalar.activation(t1[:], gps[:], AF.Copy, scale=theta)
                t2 = sb.tile([64, 128], F32, tag="t2")
                nc.vector.tensor_scalar_mul(t2[:], supT[:], 1.0 - theta)
                hT = sb.tile([64, 128], F32, tag="hTn")
                nc.vector.tensor_tensor(out=hT[:], in0=t1[:], in1=t2[:],
                                        op=ALU.add)
                hTr = sb.tile([64, 128], F32, tag="hTr")
                nc.scalar.activation(hTr[:], hT[:], AF.Relu)
                hps2 = psB.tile([128, 64], F32, tag="tpB2")
                nc.tensor.matmul(out=hps2[:], lhsT=hTr[:], rhs=i64[:],
                                 is_transpose=True)
                nc.vector.tensor_copy(out=h_all[:, t * H:(t + 1) * H],
                                      in_=hps2[:])
                xsl = xmax[:, t * H:(t + 1) * H]
                if l == 0:
                    nc.vector.tensor_copy(out=xsl, in_=hps2[:])
                else:
                    nc.vector.tensor_tensor(out=xsl, in0=xsl, in1=hps2[:],
                                            op=ALU.max)

        # ---------------- head + combine ----------------
        for t in range(NT):
            xmT = transpose_128x64(xmax[:, t * H:(t + 1) * H])
            hps = psC.tile([1, 128], F32, tag="mmC")
            nc.tensor.matmul(out=hps[:], lhsT=hw[:], rhs=xmT[:],
                             start=True, stop=True)
            r1 = sb.tile([1, 128], F32, tag="r1")
            nc.vector.tensor_scalar_add(r1[:], hps[:], hbt[:])
            mr = sb.tile([1, 128], F32, tag="mr")
            nc.sync.dma_start(mr[:], mrow_d[:, t * 128:(t + 1) * 128])
            r2 = sb.tile([1, 128], F32, tag="r2")
            nc.vector.tensor_tensor(out=r2[:], in0=r1[:], in1=mr[:],
                                    op=ALU.add)
            fr = sb.tile([1, 128], F32, tag="fr")
            nc.vector.tensor_scalar_mul(fr[:], r2[:], 0.5)
            hi_lim = min(128, NSH - t * 128)
            nc.sync.dma_start(out[:, t * 128:t * 128 + hi_lim],
                              fr[:1, 0:hi_lim])

        for _p in (psC, psB, psA, mwp, ohp, msgp, sb, big, cst):
            _p.release()

    nc.finalize()
    return nc


# ---------------------------------------------------------------- entry
def kernel(**inputs):
    x = np.asarray(inputs["x"], np.float32)
    ew = np.asarray(inputs["edge_weight"], np.float32)
    eidx = np.asarray(inputs["edge_index"])

    cores_data, g_bt, blen = _prep_edges(eidx, ew)
    nc = _build(g_bt, blen)

    rep = lambda v: np.tile(np.asarray(v, np.float32).reshape(1, -1), (128, 1))
    col = lambda v: np.asarray(v, np.float32).reshape(-1, 1)
    shared = {
        "proj_w": np.asarray(inputs["proj_w"], np.float32),
        "projb": col(inputs["proj_b"]),
        "mlp_w1": np.asarray(inputs["mlp_w1"], np.float32),
        "b1": col(inputs["mlp_b1"]),
        "mlp_w2": np.asarray(inputs["mlp_w2"], np.float32),
        "b2": col(inputs["mlp_b2"]),
        "mlp_w3": np.asarray(inputs["mlp_w3"], np.float32),
        "b3": col(inputs["mlp_b3"]),
        "head_w": np.asarray(inputs["head_w"], np.float32),
        "headb": col(inputs["head_b"]),
        "gcn_w": np.asarray(inputs["gcn_w"], np.float32),
        "ln1g": rep(inputs["ln1_g"]),
        "ln1b": rep(inputs["ln1_b"]),
        "ln2g": rep(inputs["ln2_g"]),
        "ln2b": rep(inputs["ln2_b"]),
        "diota": np.tile(np.arange(128, dtype=np.float32), (128, 1)),
    }
    in_maps = []
    for c in range(NCORES):
        idx_s, w_s, d_s = cores_data[c]
        m = dict(shared)
        xs = np.zeros((NSHP, D_IN), np.float32)
        xs[:NSH] = x[c * NSH:(c + 1) * NSH]
        m["xsh"] = xs
        for bb in range(NBUCK):
            m[f"idx{bb}"] = idx_s[bb]
            m[f"wst{bb}"] = w_s[bb]
            m[f"dst{bb}"] = d_s[bb]
        in_maps.append(m)

    import time as _time
    _t0 = _time.time()
    res = bass_utils.run_bass_kernel_spmd(
        nc, in_maps, core_ids=list(range(NCORES)))
    global LAST_EXEC_NS
    LAST_EXEC_NS = res.exec_time_ns if res.exec_time_ns else int(
        (_time.time() - _t0) * 1e9)
    outp = np.concatenate([res.results[c]["out"][0] for c in range(NCORES)])
    return outp.reshape(N, 1).astype(np.float32)



# revision 8
# speedup vs baseline: 5.7451x; 5.7451x over previous
"""JumpGCN-v2 (GCNII + JK-max + MLP branch) on 8 Trainium2 NeuronCores.

Feature-major (transposed) dataflow with hardware For_i loops to keep the
program tiny (compile + NEFF-load dominate wall-clock, not HW exec):

- nodes row-sharded 8 ways; per-layer halo = AllGather of the node-major h
  shard into a shared padded table [8*12544, 64].
- spmm per dst tile: dma_gather of h[src] rows (padded-table row ids are
  bucketed 4x so indices fit int16), one-hot (weight-folded) segment-sum
  matmuls that produce hi TRANSPOSED [64, 128] directly in PSUM.
- host folds: (1-alpha) into edge weights, alpha into h0s, theta into the
  GCN weights (W' = theta*W + (1-theta)*I), 0.5 into mlp3/head weights.
- MLP branch layernorm runs feature-major via ones-matmul partition
  reductions and an outer-product broadcast.
"""
import numpy as np

import concourse.bass as bass
import concourse.bacc as bacc
import concourse.mybir as mybir
import concourse.tile as tile
from concourse import bass_utils
from concourse.masks import make_identity

F32 = mybir.dt.float32
I16 = mybir.dt.int16
U8 = mybir.dt.uint8
AF = mybir.ActivationFunctionType
ALU = mybir.AluOpType
ds = bass.ds

NCORES = 8
N = 100000
D_IN = 128
H = 64
L = 4
ALPHA = 0.1
LAMDA = 1.0
NSH = N // NCORES            # 12500 nodes per core
NT = 98                      # dst tiles per core
NSHP = NT * 128              # 12544 padded shard rows
NP = NSHP * NCORES           # 100352 padded table rows
NBUCK = 4
BUCK = NP // NBUCK           # 25088 table rows per src bucket (< 2^15)
CH = 256                     # P-phase node chunk (2 tiles)
NCH_P = NSHP // CH           # 49
LN_EPS = 1e-5
THETA = [float(np.log(LAMDA / (l + 1) + 1.0)) for l in range(L)]
LAST_EXEC_NS = 0


# ---------------------------------------------------------------- host prep
def _prep_edges(edge_index, edge_weight):
    """Partition/pad the edge list into uniform per-(core,bucket,tile) groups
    of K 128-edge chunks. Returns per-core streams and K."""
    src = np.asarray(edge_index[0], np.int64)
    dst = np.asarray(edge_index[1], np.int64)
    w = np.asarray(edge_weight, np.float32) * (1.0 - ALPHA)

    core = dst // NSH
    dstl = dst - core * NSH
    t = dstl >> 7
    dloc = (dstl & 127).astype(np.uint8)
    srow = (src // NSH) * NSHP + (src % NSH)   # padded-table row
    b = srow // BUCK
    sidx = (srow - b * BUCK).astype(np.int16)

    key = (core * NBUCK + b) * NT + t
    counts = np.bincount(key, minlength=NCORES * NBUCK * NT)
    K = max(1, int(-(-int(counts.max()) // 128)))
    CW = K * 128

    order = np.argsort(key, kind="stable")
    gstart = np.zeros(NCORES * NBUCK * NT, np.int64)
    gstart[1:] = np.cumsum(counts)[:-1]
    skey = key[order]
    pos = np.arange(len(order)) - gstart[skey]
    spos = skey * CW + pos

    tot = NCORES * NBUCK * NT * CW
    ia = np.zeros(tot, np.int16)
    wa = np.zeros(tot, np.float32)
    da = np.zeros(tot, np.uint8)
    ia[spos] = sidx[order]
    wa[spos] = w[order]
    da[spos] = dloc[order]
    ia = ia.reshape(NCORES, -1)
    wa = wa.reshape(NCORES, -1)
    da = da.reshape(NCORES, -1)

    cores_data = []
    for c in range(NCORES):
        idx16 = np.ascontiguousarray(ia[c].reshape(-1, 16).T)    # [16, SLEN/16]
        w128 = np.ascontiguousarray(wa[c].reshape(-1, 128).T)    # [128, SLEN/128]
        d128 = np.ascontiguousarray(da[c].reshape(-1, 128).T)    # [128, SLEN/128]
        cores_data.append((idx16, w128, d128))
    return cores_data, K


# ---------------------------------------------------------------- bass build
def _build(K):
    CW = K * 128
    IW = CW // 16                 # idx cols per (bucket, tile)
    CLEN = NBUCK * NT * K         # w/dst stream cols per core
    ILEN = NBUCK * NT * IW        # idx stream cols per core

    nc = bacc.Bacc("TRN2", target_bir_lowering=False, debug=False,
                   enable_asserts=True, num_devices=NCORES)

    din = {}
    def inp(name, shape, dt=F32):
        din[name] = nc.dram_tensor(name, list(shape), dt, kind="ExternalInput")
        return din[name]

    xT_d = inp("xT", [D_IN, NSHP])
    idx16_d = inp("idx16", [16, ILEN], I16)
    dst_d = inp("dstu8", [128, CLEN], U8)
    w_d = inp("wst", [128, CLEN])
    pw_d = inp("proj_w", [D_IN, H])
    w1_d = inp("mlp_w1", [D_IN, H])
    w2_d = inp("mlp_w2", [H, H])
    gcn2_d = inp("gcn2", [L, H, H])
    w3_d = inp("w3h", [H, 1])
    hw_d = inp("hwh", [H, 1])
    cols_d = inp("cols", [H, 10])   # pbf pb01 b1 g1 be1 b2 g2 be2 ones64 pad
    row_d = inp("rowc", [1, 2 + H])  # b3h, eps, ones1[64]
    dio_d = inp("diota", [128, 128])

    idxrep = nc.dram_tensor("idxrep", [128, ILEN], I16, kind="Internal")
    bounce = nc.dram_tensor("bounce", [NSHP, H], F32, kind="Internal")
    mlrow = nc.dram_tensor("mlrow", [1, NSHP], F32, kind="Internal")
    table = nc.dram_tensor("table", [NP, H], F32, kind="Internal",
                           addr_space="Shared")
    out_d = nc.dram_tensor("out", [1, NSHP], F32, kind="ExternalOutput")

    with tile.TileContext(nc) as tc:
        cst = tc.alloc_tile_pool(name="cst", bufs=1)
        big = tc.alloc_tile_pool(name="big", bufs=1)
        ep = tc.alloc_tile_pool(name="ep", bufs=2)
        gp = tc.alloc_tile_pool(name="gp", bufs=3)
        psA = tc.alloc_tile_pool(name="psA", bufs=2, space="PSUM")
        psB = tc.alloc_tile_pool(name="psB", bufs=2, space="PSUM")
        psC = tc.alloc_tile_pool(name="psC", bufs=2, space="PSUM")

        i64 = cst.tile([64, 64], F32)
        make_identity(nc, i64[:])
        dio = cst.tile([128, 128], F32)
        nc.sync.dma_start(dio[:], dio_d[:, :])

        def load_const(t_, shape=None):
            tl = cst.tile(shape or list(t_.shape), t_.dtype, tag=t_.name)
            nc.sync.dma_start(tl[:], t_[:, :])
            return tl

        pw = load_const(pw_d)
        w1t = load_const(w1_d)
        w2t = load_const(w2_d)
        w3t = load_const(w3_d)
        hwt = load_const(hw_d)
        colst = load_const(cols_d)
        rowct = load_const(row_d)
        gwt = []
        for l in range(L):
            g_ = cst.tile([H, H], F32, tag=f"gw{l}", name=f"gw{l}")
            nc.sync.dma_start(g_[:], gcn2_d[l, :, :])
            gwt.append(g_)

        pbft = colst[:, 0:1]
        pb01t = colst[:, 1:2]
        b1t = colst[:, 2:3]
        g1t = colst[:, 3:4]
        be1t = colst[:, 4:5]
        b2t = colst[:, 5:6]
        g2t = colst[:, 6:7]
        be2t = colst[:, 7:8]
        o64t = colst[:, 8:9]
        b3ht = rowct[:, 0:1]
        epst = rowct[:, 1:2]
        o1t = rowct[:, 2:2 + H]

        # resident edge streams
        d8 = big.tile([128, CLEN], U8)
        nc.sync.dma_start(d8[:], dst_d[:, :])
        dstf = big.tile([128, CLEN], F32)
        nc.vector.tensor_copy(out=dstf[:], in_=d8[:])
        wstt = big.tile([128, CLEN], F32)
        nc.sync.dma_start(wstt[:], w_d[:, :])
        # replicate idx 16 -> 128 partitions in DRAM
        for k_ in range(8):
            nc.sync.dma_start(idxrep[k_ * 16:(k_ + 1) * 16, :], idx16_d[:, :])

        h0sT = big.tile([H, NSHP], F32)
        xmaxT = big.tile([H, NSHP], F32)
        nc.vector.memset(xmaxT[:], 0.0)

        bounce_v = bounce.rearrange("(t p) h -> p t h", p=128)
        idxrep_v = idxrep.rearrange("p (b c) -> p b c", b=NBUCK)
        dst_v = dstf[:].rearrange("p (b c) -> p b c", b=NBUCK)
        w_v = wstt[:].rearrange("p (b c) -> p b c", b=NBUCK)

        def ln_relu_fm(mp, bct, gt, bet):
            """feature-major LN+affine+relu on PSUM [H, CH] -> SBUF [H, CH]"""
            m = ep.tile([H, CH], F32, tag="lnm")
            nc.vector.tensor_scalar_add(m[:], mp[:], bct)
            sq = ep.tile([H, CH], F32, tag="lnsq")
            nc.scalar.activation(sq[:], mp[:], AF.Square, bias=bct)
            s12 = psB.tile([1, 2 * CH], F32, tag="lnr")
            nc.tensor.matmul(out=s12[:, 0:CH], lhsT=o64t, rhs=m[:],
                             start=True, stop=True)
            nc.tensor.matmul(out=s12[:, CH:2 * CH], lhsT=o64t, rhs=sq[:],
                             start=True, stop=True)
            mu = ep.tile([1, CH], F32, tag="lnmu")
            nc.scalar.activation(mu[:], s12[:, 0:CH], AF.Copy, scale=1.0 / H)
            mu2 = ep.tile([1, CH], F32, tag="lnmu2")
            nc.vector.tensor_tensor(out=mu2[:], in0=mu[:], in1=mu[:],
                                    op=ALU.mult)
            vr = ep.tile([1, CH], F32, tag="lnvr")
            nc.scalar.activation(vr[:], s12[:, CH:2 * CH], AF.Copy,
                                 scale=1.0 / H)
            vr2 = ep.tile([1, CH], F32, tag="lnvr2")
            nc.vector.tensor_tensor(out=vr2[:], in0=vr[:], in1=mu2[:],
                                    op=ALU.subtract)
            sd = ep.tile([1, CH], F32, tag="lnsd")
            nc.scalar.activation(sd[:], vr2[:], AF.Sqrt, bias=epst)
            rs = ep.tile([1, CH], F32, tag="lnrs")
            nc.vector.reciprocal(rs[:], sd[:])
            a = ep.tile([1, CH], F32, tag="lna")
            nc.vector.tensor_tensor(out=a[:], in0=mu[:], in1=rs[:],
                                    op=ALU.mult)
            bb = psB.tile([H, 2 * CH], F32, tag="lnbb")
            nc.tensor.matmul(out=bb[:, 0:CH], lhsT=o1t, rhs=rs[:],
                             start=True, stop=True)
            nc.tensor.matmul(out=bb[:, CH:2 * CH], lhsT=o1t, rhs=a[:],
                             start=True, stop=True)
            z = ep.tile([H, CH], F32, tag="lnz")
            nc.vector.tensor_tensor(out=z[:], in0=m[:], in1=bb[:, 0:CH],
                                    op=ALU.mult)
            z2 = ep.tile([H, CH], F32, tag="lnz2")
            nc.vector.tensor_tensor(out=z2[:], in0=z[:], in1=bb[:, CH:2 * CH],
                                    op=ALU.subtract)
            y = ep.tile([H, CH], F32, tag="lny")
            nc.scalar.activation(y[:], z2[:], AF.Relu, scale=gt, bias=bet)
            return y

        # ---------------- phase P: proj + MLP branch (For_i over 49 chunks)
        def pbody(q):
            xt = ep.tile([D_IN, CH], F32, tag="xt")
            nc.sync.dma_start(xt[:], xT_d[:, ds(q * CH, CH)])
            pp = psA.tile([H, CH], F32, tag="mmA")
            nc.tensor.matmul(out=pp[:], lhsT=pw[:], rhs=xt[:],
                             start=True, stop=True)
            nc.vector.tensor_scalar(out=h0sT[:, ds(q * CH, CH)], in0=pp[:],
                                    scalar1=ALPHA, scalar2=pb01t,
                                    op0=ALU.mult, op1=ALU.add)
            h0f = ep.tile([H, CH], F32, tag="h0f")
            nc.vector.tensor_scalar_add(h0f[:], pp[:], pbft)
            tp = psC.tile([128, 2 * H], F32, tag="tp")
            nc.tensor.transpose(out=tp[:, 0:H], in_=h0f[:, 0:128],
                                identity=i64[:])
            nc.tensor.transpose(out=tp[:, H:2 * H], in_=h0f[:, 128:256],
                                identity=i64[:])
            tps = ep.tile([128, 2 * H], F32, tag="tps")
            nc.vector.tensor_copy(out=tps[:], in_=tp[:])
            nc.sync.dma_start(
                bounce_v[:, ds(q * 2, 2), :],
                tps[:].rearrange("p (t h) -> p t h", h=H))

            m1p = psA.tile([H, CH], F32, tag="mmA")
            nc.tensor.matmul(out=m1p[:], lhsT=w1t[:], rhs=xt[:],
                             start=True, stop=True)
            y1 = ln_relu_fm(m1p, b1t, g1t, be1t)
            m2p = psA.tile([H, CH], F32, tag="mmA")
            nc.tensor.matmul(out=m2p[:], lhsT=w2t[:], rhs=y1[:],
                             start=True, stop=True)
            y2 = ln_relu_fm(m2p, b2t, g2t, be2t)
            m3p = psB.tile([1, 2 * CH], F32, tag="lnr")
            nc.tensor.matmul(out=m3p[:, 0:CH], lhsT=w3t[:], rhs=y2[:],
                             start=True, stop=True)
            brow = ep.tile([1, CH], F32, tag="brow")
            nc.vector.tensor_scalar_add(brow[:], m3p[:, 0:CH], b3ht)
            nc.sync.dma_start(mlrow[:, ds(q * CH, CH)], brow[:])

        tc.For_i_unrolled(0, NCH_P, 1, pbody, max_unroll=7)

        # ---------------- GCN layers ----------------
        rg = [list(range(NCORES))]
        nc.gpsimd.collective_compute(
            "AllGather", ALU.bypass, replica_groups=rg,
            ins=[bounce[:, :]], outs=[table[:, :]])

        for l in range(L):
            last = l == L - 1

            def lbody(i, l=l, last=last):
                idxs = gp.tile([128, NBUCK, IW], I16, tag="idxs")
                nc.sync.dma_start(idxs[:], idxrep_v[:, :, ds(i * IW, IW)])
                mgs = []
                for b2 in range(NBUCK):
                    mg = gp.tile([128, K, H], F32, tag=f"mg{b2}")
                    nc.gpsimd.dma_gather(
                        mg[:, :, :], table[b2 * BUCK:(b2 + 1) * BUCK, :],
                        idxs[:, b2, :], CW, CW, H, elem_step=H,
                        single_packet=False)
                    mgs.append(mg)
                acc = psA.tile([H, 128], F32, tag="mmA")
                for b2 in range(NBUCK):
                    ohw = gp.tile([128, K, 128], F32, tag="ohw")
                    nc.vector.tensor_tensor(
                        out=ohw[:],
                        in0=dst_v[:, b2, ds(i * K, K)].unsqueeze(2)
                            .to_broadcast([128, K, 128]),
                        in1=dio[:].unsqueeze(1).to_broadcast([128, K, 128]),
                        op=ALU.is_equal)
                    nc.vector.tensor_tensor(
                        out=ohw[:], in0=ohw[:],
                        in1=w_v[:, b2, ds(i * K, K)].unsqueeze(2)
                            .to_broadcast([128, K, 128]),
                        op=ALU.mult)
                    for j in range(K):
                        nc.tensor.matmul(
                            out=acc[:], lhsT=mgs[b2][:, j, :],
                            rhs=ohw[:, j, :],
                            start=(b2 == 0 and j == 0),
                            stop=(b2 == NBUCK - 1 and j == K - 1))
                sup = ep.tile([H, 128], F32, tag="sup")
                nc.vector.tensor_tensor(out=sup[:], in0=acc[:],
                                        in1=h0sT[:, ds(i * 128, 128)],
                                        op=ALU.add)
                g_ = psA.tile([H, 128], F32, tag="mmA")
                nc.tensor.matmul(out=g_[:], lhsT=gwt[l][:], rhs=sup[:],
                                 start=True, stop=True)
                hT = ep.tile([H, 128], F32, tag="hT")
                nc.scalar.activation(hT[:], g_[:], AF.Relu)
                xsl = xmaxT[:, ds(i * 128, 128)]
                nc.vector.tensor_tensor(out=xsl, in0=xsl, in1=hT[:],
                                        op=ALU.max)
                if not last:
                    tp = psC.tile([128, 2 * H], F32, tag="tp")
                    nc.tensor.transpose(out=tp[:, 0:H], in_=hT[:],
                                        identity=i64[:])
                    tps = ep.tile([128, H], F32, tag="tpl")
                    nc.vector.tensor_copy(out=tps[:], in_=tp[:, 0:H])
                    nc.sync.dma_start(
                        bounce_v[:, ds(i, 1), :],
                        tps[:].rearrange("p (o h) -> p o h", o=1))
                else:
                    hd = psC.tile([1, 128], F32, tag="tp")
                    nc.tensor.matmul(out=hd[:], lhsT=hwt[:], rhs=xsl,
                                     start=True, stop=True)
                    mrow = ep.tile([1, 128], F32, tag="mrow")
                    nc.sync.dma_start(mrow[:], mlrow[:, ds(i * 128, 128)])
                    orow = ep.tile([1, 128], F32, tag="orow")
                    nc.vector.tensor_tensor(out=orow[:], in0=hd[:],
                                            in1=mrow[:], op=ALU.add)
                    nc.sync.dma_start(out_d[:, ds(i * 128, 128)], orow[:])

            tc.For_i_unrolled(0, NT, 1, lbody, max_unroll=7)
            if not last:
                nc.gpsimd.collective_compute(
                    "AllGather", ALU.bypass, replica_groups=rg,
                    ins=[bounce[:, :]], outs=[table[:, :]])

        for _p in (psC, psB, psA, gp, ep, big, cst):
            _p.release()

    nc.finalize()
    return nc


# ---------------------------------------------------------------- entry
def kernel(**inputs):
    x = np.asarray(inputs["x"], np.float32)
    ew = np.asarray(inputs["edge_weight"], np.float32)
    eidx = np.asarray(inputs["edge_index"])

    cores_data, K = _prep_edges(eidx, ew)
    nc = _build(K)

    col = lambda v: np.asarray(v, np.float32).reshape(-1, 1)
    gcn_w = np.asarray(inputs["gcn_w"], np.float32)
    gcn2 = np.stack([
        THETA[l] * gcn_w[l] + (1.0 - THETA[l]) * np.eye(H, dtype=np.float32)
        for l in range(L)
    ])
    cols = np.zeros((H, 10), np.float32)
    cols[:, 0:1] = col(inputs["proj_b"])
    cols[:, 1:2] = ALPHA * col(inputs["proj_b"])
    cols[:, 2:3] = col(inputs["mlp_b1"])
    cols[:, 3:4] = col(inputs["ln1_g"])
    cols[:, 4:5] = col(inputs["ln1_b"])
    cols[:, 5:6] = col(inputs["mlp_b2"])
    cols[:, 6:7] = col(inputs["ln2_g"])
    cols[:, 7:8] = col(inputs["ln2_b"])
    cols[:, 8:9] = 1.0
    rowc = np.zeros((1, 2 + H), np.float32)
    rowc[0, 0] = 0.5 * (float(np.asarray(inputs["mlp_b3"]).reshape(-1)[0])
                        + float(np.asarray(inputs["head_b"]).reshape(-1)[0]))
    rowc[0, 1] = LN_EPS
    rowc[0, 2:] = 1.0

    shared = {
        "proj_w": np.asarray(inputs["proj_w"], np.float32),
        "mlp_w1": np.asarray(inputs["mlp_w1"], np.float32),
        "mlp_w2": np.asarray(inputs["mlp_w2"], np.float32),
        "gcn2": gcn2,
        "w3h": 0.5 * np.asarray(inputs["mlp_w3"], np.float32),
        "hwh": 0.5 * np.asarray(inputs["head_w"], np.float32),
        "cols": cols,
        "rowc": rowc,
        "diota": np.tile(np.arange(128, dtype=np.float32), (128, 1)),
    }
    in_maps = []
    for c in range(NCORES):
        idx16, w128, d128 = cores_data[c]
        m = dict(shared)
        xs = np.zeros((D_IN, NSHP), np.float32)
        xs[:, :NSH] = x[c * NSH:(c + 1) * NSH].T
        m["xT"] = xs
        m["idx16"] = idx16
        m["dstu8"] = d128
        m["wst"] = w128
        in_maps.append(m)

    import time as _time
    _t0 = _time.time()
    res = bass_utils.run_bass_kernel_spmd(
        nc, in_maps, core_ids=list(range(NCORES)))
    global LAST_EXEC_NS
    LAST_EXEC_NS = res.exec_time_ns if res.exec_time_ns else int(
        (_time.time() - _t0) * 1e9)
    outp = np.concatenate(
        [res.results[c]["out"][0, :NSH] for c in range(NCORES)])
    return outp.reshape(N, 1).astype(np.float32)
